# revision 7
# baseline (speedup 1.0000x reference)
"""DCNv2 (spatially-constant offsets) Trainium2 Bass kernel, 8-core SPMD.

Math: out[B,g*16+o,i,j] = sum_{ky,kx,c} w[g,o,c,ky,kx] * smp
     smp = bilinear sample of x[B//2, g*3+c] at (i + dy(ky), j + dx(kx)),
     dy = p[ky]*(1+3/off_y), dx = p[kx]*(1+3/off_x), p = [-1,0,1],
     zero padding outside the image.

Separable structure: integer y-shift + y-fraction folded into a per-partition
row GATHER (indirect DMA) over a zero-padded input; integer x-shift +
x-fraction folded into a small set of PSUM-accumulating matmuls whose rhs
reads the gathered rows at a per-pass column offset delta, with host-folded
block-diagonal weights (data, so the single SPMD program works for all
cores' different offsets).

Sharding: off_b (16) split 2-per-core across 8 cores (core i handles
off_b {2i, 2i+1}, which both read input batch i).
"""

import os
import sys

sys.path.insert(0, "/opt/trn_rl_repo")

import numpy as np

import concourse.bass as bass
import concourse.bacc as bacc
import concourse.mybir as mybir
from concourse.tile import TileContext
from concourse.bass_utils import run_bass_kernel_spmd

# ---- fixed problem geometry (hardcoded per task rules) ----
KS = 3
H = W = 160
PAD = 5
HP = WP = H + 2 * PAD  # 170
CH = 9                  # channels per input batch (num_sq*ct)
G = 3                   # groups
CG = 3                  # channels per group
COUT = 48
OG = COUT // G          # 16 outputs per group
NCORES = 8
NPAIR = 6               # (2 off_b) x (3 groups) per core
# y-tap slots (ky, a): ky=1 is the center (dy=0 -> single tap)
YT = [(0, 0), (0, 1), (1, 0), (2, 0), (2, 1)]
NT = len(YT)            # 5
KPART = NPAIR * NT * CG  # 90 contraction partitions
M = NPAIR * OG           # 96 output partitions
PR = np.array([-1.0, 0.0, 1.0], dtype=np.float64)

_prog_cache = {}


# ---------------------------------------------------------------- device code
def _build_program(ndelta, deltas):
    """One SPMD program; per-core variation only through tensor data."""
    nc = bacc.Bacc("TRN2", target_bir_lowering=False, debug=False,
                   num_devices=NCORES)
    xpad = nc.declare_dram_parameter("xpad", [CH * HP, WP], mybir.dt.float32,
                                     isOutput=False)
    gidx = nc.declare_dram_parameter("gidx", [KPART, H], mybir.dt.int32,
                                     isOutput=False)
    wfold = nc.declare_dram_parameter("wfold", [KPART, ndelta * M],
                                      mybir.dt.float32, isOutput=False)
    y = nc.declare_dram_parameter("y", [M, H, W], mybir.dt.float32,
                                  isOutput=True)

    SH = 32  # gather strip height (output rows per gather)
    strips = [(i0, min(SH, H - i0)) for i0 in range(0, H, SH)]

    with TileContext(nc) as tc:
        with (
            tc.tile_pool(name="const", bufs=1) as cpool,
            tc.tile_pool(name="gat", bufs=3) as gpool,
            tc.tile_pool(name="ps", bufs=4, space="PSUM") as ppool,
            tc.tile_pool(name="ost", bufs=4) as opool,
        ):
            idx_sb = cpool.tile([KPART, H], mybir.dt.int32, tag="idx")
            nc.sync.dma_start(idx_sb[:], gidx[:])
            w_sb = cpool.tile([KPART, ndelta * M], mybir.dt.float32, tag="w")
            nc.sync.dma_start(w_sb[:], wfold[:])

            for (i0, sh) in strips:
                gt = gpool.tile([KPART, SH, WP], mybir.dt.float32, tag="g")
                # HW indirect DMA honors exactly one offset per partition
                # (multi-unit offsets are consumed in a broken order), so
                # gather one strip row per instruction.
                for r in range(sh):
                    nc.gpsimd.indirect_dma_start(
                        out=gt[:, r, :],
                        out_offset=None,
                        in_=xpad[:],
                        in_offset=bass.IndirectOffsetOnAxis(
                            ap=idx_sb[:, i0 + r:i0 + r + 1], axis=0),
                    )
                for il in range(0, sh, 3):
                    rows = min(3, sh - il)
                    n = rows * W
                    pt = ppool.tile([M, 512], mybir.dt.float32, tag="p")
                    for d in range(ndelta):
                        off = PAD + deltas[d]
                        nc.tensor.matmul(
                            pt[:, :n],
                            w_sb[:, d * M:(d + 1) * M],
                            gt[:, il:il + rows, off:off + W],
                            start=(d == 0),
                            stop=(d == ndelta - 1),
                        )
                    ot = opool.tile([M, 512], mybir.dt.float32, tag="o")
                    nc.scalar.copy(ot[:, :n], pt[:, :n])
                    nc.sync.dma_start(
                        y[:, i0 + il:i0 + il + rows, :], ot[:, :n])
    nc.finalize()
    return nc


# ------------------------------------------------------------------ host prep
def _fold(inputs):
    """Per-core in_maps + the program-wide delta list."""
    x = np.asarray(inputs["input"], dtype=np.float32)    # (8,1,9,160,160)
    wt = np.asarray(inputs["weight"], dtype=np.float32)  # (3,3,48,3)
    off = np.asarray(inputs["offset"], dtype=np.float64)  # (16,3,2)

    # wmat[g, o, c, k]  (k = ky*3+kx)
    wmat = wt.transpose(2, 3, 0, 1).reshape(G, OG, CG, KS * KS)

    d_y = 1.0 + KS / off[:, :, 0]   # (16,3)
    d_x = 1.0 + KS / off[:, :, 1]
    dy = PR[None, None, :] * d_y[:, :, None]   # (16,3,ky)
    dx = PR[None, None, :] * d_x[:, :, None]
    oy = np.floor(dy).astype(np.int64)
    ox = np.floor(dx).astype(np.int64)
    wy = dy - oy
    wx = dx - ox

    # program-wide union of x-column offsets (delta = ox + b)
    dset = set()
    for B in range(off.shape[0]):
        for g in range(G):
            for kx in range(KS):
                dset.add(int(ox[B, g, kx]))
                if wx[B, g, kx] != 0.0:
                    dset.add(int(ox[B, g, kx]) + 1)
    deltas = sorted(dset)
    nd = len(deltas)
    dpos = {d: i for i, d in enumerate(deltas)}

    in_maps = []
    for core in range(NCORES):
        xc = x[core, 0]  # (9,160,160)
        xp = np.zeros((CH, HP, WP), dtype=np.float32)
        xp[:, PAD:PAD + H, PAD:PAD + W] = xc

        gi = np.zeros((KPART, H), dtype=np.int32)
        wf = np.zeros((KPART, nd, M), dtype=np.float32)
        for p in range(2):
            B = 2 * core + p
            for g in range(G):
                q = p * G + g
                for t, (ky, a) in enumerate(YT):
                    cy = (1.0 - wy[B, g, ky]) if a == 0 else wy[B, g, ky]
                    yshift = int(oy[B, g, ky]) + a
                    for c in range(CG):
                        P = (q * NT + t) * CG + c
                        ch = g * CG + c
                        gi[P, :] = (ch * HP + PAD + yshift
                                    + np.arange(H, dtype=np.int32))
                        for kx in range(KS):
                            for b in range(2):
                                cx = (1.0 - wx[B, g, kx]) if b == 0 \
                                    else wx[B, g, kx]
                                if cx == 0.0:
                                    continue
                                d = dpos[int(ox[B, g, kx]) + b]
                                k = ky * KS + kx
                                wf[P, d, q * OG:(q + 1) * OG] += (
                                    cy * cx * wmat[g, :, c, k]
                                ).astype(np.float32)
        in_maps.append({
            "xpad": xp.reshape(CH * HP, WP),
            "gidx": gi,
            "wfold": wf.reshape(KPART, nd * M),
        })
    return in_maps, deltas


def kernel(**inputs):
    in_maps, deltas = _fold(inputs)
    nd = len(deltas)
    key = (nd, tuple(deltas))
    if key not in _prog_cache:
        _prog_cache[key] = _build_program(nd, deltas)
    nc = _prog_cache[key]

    trace = bool(int(os.environ.get("BASSDCN_TRACE", "0")))
    if trace:
        _install_ntff_hook()
    res = run_bass_kernel_spmd(nc, in_maps, list(range(NCORES)), trace=trace)
    if trace:
        kernel.last_exec_time_ns = res.exec_time_ns
        kernel.last_results = res

    out = np.empty((16, COUT, H, W), dtype=np.float32)
    for core in range(NCORES):
        yc = res.results[core]["y"].reshape(2, G, OG, H, W)
        out[2 * core] = yc[0].reshape(COUT, H, W)
        out[2 * core + 1] = yc[1].reshape(COUT, H, W)
    return out


def _install_ntff_hook():
    """The agent image's antenv lacks axon_hooks; synthesize it so
    run_bass_kernel_spmd(trace=True) can NTFF-profile via libaxon_pjrt."""
    import types
    try:
        import antenv.axon_hooks  # noqa: F401
        return
    except ImportError:
        pass
    try:
        sys.path.insert(0, "/root/.axon_site")
        from trn_agent_boot.trn_boot import _ntff_profile_via_ctypes
        hook = _ntff_profile_via_ctypes("/opt/axon/libaxon_pjrt.so")
    except Exception:
        hook = None
    m = types.ModuleType("antenv.axon_hooks")
    m.get_axon_ntff_profile_hook = lambda: hook
    m.set_axon_ntff_profile_hook = lambda h: None
    sys.modules["antenv.axon_hooks"] = m


# ------------------------------------------------- tile drain walrus workaround
def _patch_tile_drain():
    from bass_rust import ScopedClock

    def _patched(self, tick_clock, wait_clock):
        nc = self.nc
        drain_inst = nc.sync.drain()
        wait_clock.add_sem_waits(
            drain_inst.ins, ScopedClock({None: tick_clock.global_clock}))
        si = drain_inst.ins.sync_info
        waits = list(si.on_wait or [])
        if len(waits) > 1:
            si.on_wait = waits[:1]
            drain_inst.ins.sync_info = si
            for w in waits[1:]:
                nop = nc.sync.nop(nofuse=True, hint="drain_wait_split")
                nsi = nop.ins.sync_info
                if nsi is None:
                    nsi = mybir.SyncInfo(on_wait=[w], on_update=[])
                else:
                    nsi.on_wait = [w]
                nop.ins.sync_info = nsi
        nc.all_engine_barrier()
        assert self.sems is not None
        popped = nc._tile_sem_poison_stack.pop()
        assert popped is self._sem_poison
        nc.clear_and_free_semaphores(list(self.sems.allocated().values()))
        nc.all_engine_barrier()

    TileContext._drain_and_barrier = _patched


_patch_tile_drain()


# revision 11
# speedup vs baseline: 3.6032x; 3.6032x over previous
"""DCNv2 (spatially-constant offsets) Trainium2 Bass kernel, 8-core SPMD.

Math: out[B,g*16+o,i,j] = sum_{ky,kx,c} w[g,o,c,ky,kx] * smp
     smp = bilinear sample of x[B//2, g*3+c] at (i + dy(ky), j + dx(kx)),
     dy = p[ky]*(1+3/off_y), dx = p[kx]*(1+3/off_x), p = [-1,0,1],
     zero padding outside the image.

Separable structure: integer y-shift + y-fraction folded into a per-partition
row GATHER (indirect DMA) over a zero-padded input; integer x-shift +
x-fraction folded into a small set of PSUM-accumulating matmuls whose rhs
reads the gathered rows at a per-pass column offset delta, with host-folded
block-diagonal weights (data, so the single SPMD program works for all
cores' different offsets).

Sharding: off_b (16) split 2-per-core across 8 cores (core i handles
off_b {2i, 2i+1}, which both read input batch i).
"""

import os
import sys

sys.path.insert(0, "/opt/trn_rl_repo")

import ml_dtypes
import numpy as np

import concourse.bass as bass
import concourse.bacc as bacc
import concourse.mybir as mybir
from concourse.tile import TileContext
from concourse.bass_utils import run_bass_kernel_spmd

# ---- fixed problem geometry (hardcoded per task rules) ----
KS = 3
H = W = 160
PAD = 5
HP = WP = H + 2 * PAD  # 170
CH = 9                  # channels per input batch (num_sq*ct)
G = 3                   # groups
CG = 3                  # channels per group
COUT = 48
OG = COUT // G          # 16 outputs per group
NCORES = 8
NPAIR = 6               # (2 off_b) x (3 groups) per core
# y-tap slots (ky, a): ky=1 is the center (dy=0 -> single tap)
YT = [(0, 0), (0, 1), (1, 0), (2, 0), (2, 1)]
NT = len(YT)            # 5
KPART = NPAIR * NT * CG  # 90 contraction partitions
M = NPAIR * OG           # 96 output partitions
PR = np.array([-1.0, 0.0, 1.0], dtype=np.float64)

# fat-row gather table: channel rows padded to HPAD, strip height SH rows per
# gathered unit, one alignment copy per possible y-start s in [1, 9]
SH = 32
HPAD = 192               # 6*SH, multiple of SH covering HP=170
NS = 9                   # alignment copies (start = i0 + 5 + yshift, yshift in [-4,4])
FATROWS = (CH * HPAD - SH) // SH  # fat rows per alignment copy = 53
FATW = SH * WP           # 5440 elements per fat row

_prog_cache = {}


# ---------------------------------------------------------------- device code
def _build_program(ndelta, deltas):
    """One SPMD program; per-core variation only through tensor data."""
    nc = bacc.Bacc("TRN2", target_bir_lowering=False, debug=False,
                   num_devices=NCORES)
    xtab = nc.declare_dram_parameter("xtab", [NS * FATROWS, FATW],
                                     mybir.dt.bfloat16, isOutput=False)
    gidx = nc.declare_dram_parameter("gidx", [KPART, H // SH], mybir.dt.int32,
                                     isOutput=False)
    wfold = nc.declare_dram_parameter("wfold", [KPART, ndelta * M],
                                      mybir.dt.bfloat16, isOutput=False)
    y = nc.declare_dram_parameter("y", [M, H, W], mybir.dt.float32,
                                  isOutput=True)

    nstrips = H // SH

    with TileContext(nc) as tc:
        with (
            tc.tile_pool(name="const", bufs=1) as cpool,
            tc.tile_pool(name="gat", bufs=3) as gpool,
            tc.tile_pool(name="ps", bufs=4, space="PSUM") as ppool,
            tc.tile_pool(name="ost", bufs=4) as opool,
        ):
            idx_sb = cpool.tile([KPART, nstrips], mybir.dt.int32, tag="idx")
            nc.sync.dma_start(idx_sb[:], gidx[:])
            w_sb = cpool.tile([KPART, ndelta * M], mybir.dt.bfloat16, tag="w")
            nc.sync.dma_start(w_sb[:], wfold[:])

            for s in range(nstrips):
                i0 = s * SH
                # one fat-row gather per strip: partition p receives SH
                # consecutive padded rows starting at its (y-shifted) base
                gt = gpool.tile([KPART, SH, WP], mybir.dt.bfloat16, tag="g")
                nc.gpsimd.indirect_dma_start(
                    out=gt.rearrange("p a b -> p (a b)"),
                    out_offset=None,
                    in_=xtab[:],
                    in_offset=bass.IndirectOffsetOnAxis(
                        ap=idx_sb[:, s:s + 1], axis=0),
                )
                for il in range(0, SH, 3):
                    rows = min(3, SH - il)
                    n = rows * W
                    pt = ppool.tile([M, 512], mybir.dt.float32, tag="p")
                    for d in range(ndelta):
                        off = PAD + deltas[d]
                        nc.tensor.matmul(
                            pt[:, :n],
                            w_sb[:, d * M:(d + 1) * M],
                            gt[:, il:il + rows, off:off + W],
                            start=(d == 0),
                            stop=(d == ndelta - 1),
                        )
                    ot = opool.tile([M, 512], mybir.dt.float32, tag="o")
                    nc.scalar.copy(ot[:, :n], pt[:, :n])
                    nc.sync.dma_start(
                        y[:, i0 + il:i0 + il + rows, :], ot[:, :n])
    nc.finalize()
    return nc


# ------------------------------------------------------------------ host prep
def _fold(inputs):
    """Per-core in_maps + the program-wide delta list."""
    x = np.asarray(inputs["input"], dtype=np.float32)    # (8,1,9,160,160)
    wt = np.asarray(inputs["weight"], dtype=np.float32)  # (3,3,48,3)
    off = np.asarray(inputs["offset"], dtype=np.float64)  # (16,3,2)

    # wmat[g, o, c, k]  (k = ky*3+kx)
    wmat = wt.transpose(2, 3, 0, 1).reshape(G, OG, CG, KS * KS)

    d_y = 1.0 + KS / off[:, :, 0]   # (16,3)
    d_x = 1.0 + KS / off[:, :, 1]
    dy = PR[None, None, :] * d_y[:, :, None]   # (16,3,ky)
    dx = PR[None, None, :] * d_x[:, :, None]
    oy = np.floor(dy).astype(np.int64)
    ox = np.floor(dx).astype(np.int64)
    wy = dy - oy
    wx = dx - ox

    # program-wide union of x-column offsets (delta = ox + b)
    dset = set()
    for B in range(off.shape[0]):
        for g in range(G):
            for kx in range(KS):
                dset.add(int(ox[B, g, kx]))
                if wx[B, g, kx] != 0.0:
                    dset.add(int(ox[B, g, kx]) + 1)
    deltas = sorted(dset)
    nd = len(deltas)
    dpos = {d: i for i, d in enumerate(deltas)}

    nstrips = H // SH
    in_maps = []
    for core in range(NCORES):
        xc = x[core, 0]  # (9,160,160)
        xp = np.zeros((CH, HPAD, WP), dtype=ml_dtypes.bfloat16)
        xp[:, PAD:PAD + H, PAD:PAD + W] = xc.astype(ml_dtypes.bfloat16)
        xflat = xp.reshape(CH * HPAD, WP)
        xtab = np.stack([
            xflat[s:s + FATROWS * SH].reshape(FATROWS, FATW)
            for s in range(1, NS + 1)
        ]).reshape(NS * FATROWS, FATW)

        gi = np.zeros((KPART, nstrips), dtype=np.int32)
        wf = np.zeros((KPART, nd, M), dtype=np.float64)
        for p in range(2):
            B = 2 * core + p
            for g in range(G):
                q = p * G + g
                for t, (ky, a) in enumerate(YT):
                    cy = (1.0 - wy[B, g, ky]) if a == 0 else wy[B, g, ky]
                    s_al = PAD + int(oy[B, g, ky]) + a  # in [1, 9]
                    for c in range(CG):
                        P = (q * NT + t) * CG + c
                        ch = g * CG + c
                        for st in range(nstrips):
                            gi[P, st] = ((s_al - 1) * FATROWS
                                         + (ch * HPAD + st * SH) // SH)
                        for kx in range(KS):
                            for b in range(2):
                                cx = (1.0 - wx[B, g, kx]) if b == 0 \
                                    else wx[B, g, kx]
                                if cx == 0.0:
                                    continue
                                d = dpos[int(ox[B, g, kx]) + b]
                                k = ky * KS + kx
                                wf[P, d, q * OG:(q + 1) * OG] += (
                                    cy * cx * wmat[g, :, c, k])
        in_maps.append({
            "xtab": xtab,
            "gidx": gi,
            "wfold": wf.reshape(KPART, nd * M).astype(ml_dtypes.bfloat16),
        })
    return in_maps, deltas


def kernel(**inputs):
    in_maps, deltas = _fold(inputs)
    nd = len(deltas)
    key = (nd, tuple(deltas))
    if key not in _prog_cache:
        _prog_cache[key] = _build_program(nd, deltas)
    nc = _prog_cache[key]

    trace = bool(int(os.environ.get("BASSDCN_TRACE", "0")))
    if trace:
        _install_ntff_hook()
    res = run_bass_kernel_spmd(nc, in_maps, list(range(NCORES)), trace=trace)
    if trace:
        kernel.last_exec_time_ns = res.exec_time_ns
        kernel.last_results = res

    out = np.empty((16, COUT, H, W), dtype=np.float32)
    for core in range(NCORES):
        yc = res.results[core]["y"].reshape(2, G, OG, H, W)
        out[2 * core] = yc[0].reshape(COUT, H, W)
        out[2 * core + 1] = yc[1].reshape(COUT, H, W)
    return out


def _install_ntff_hook():
    """The agent image's antenv lacks axon_hooks; synthesize it so
    run_bass_kernel_spmd(trace=True) can NTFF-profile via libaxon_pjrt."""
    import types
    try:
        import antenv.axon_hooks  # noqa: F401
        return
    except ImportError:
        pass
    try:
        sys.path.insert(0, "/root/.axon_site")
        from trn_agent_boot.trn_boot import _ntff_profile_via_ctypes
        hook = _ntff_profile_via_ctypes("/opt/axon/libaxon_pjrt.so")
    except Exception:
        hook = None
    m = types.ModuleType("antenv.axon_hooks")
    m.get_axon_ntff_profile_hook = lambda: hook
    m.set_axon_ntff_profile_hook = lambda h: None
    sys.modules["antenv.axon_hooks"] = m


# ------------------------------------------------- tile drain walrus workaround
def _patch_tile_drain():
    from bass_rust import ScopedClock

    def _patched(self, tick_clock, wait_clock):
        nc = self.nc
        drain_inst = nc.sync.drain()
        wait_clock.add_sem_waits(
            drain_inst.ins, ScopedClock({None: tick_clock.global_clock}))
        si = drain_inst.ins.sync_info
        waits = list(si.on_wait or [])
        if len(waits) > 1:
            si.on_wait = waits[:1]
            drain_inst.ins.sync_info = si
            for w in waits[1:]:
                nop = nc.sync.nop(nofuse=True, hint="drain_wait_split")
                nsi = nop.ins.sync_info
                if nsi is None:
                    nsi = mybir.SyncInfo(on_wait=[w], on_update=[])
                else:
                    nsi.on_wait = [w]
                nop.ins.sync_info = nsi
        nc.all_engine_barrier()
        assert self.sems is not None
        popped = nc._tile_sem_poison_stack.pop()
        assert popped is self._sem_poison
        nc.clear_and_free_semaphores(list(self.sems.allocated().values()))
        nc.all_engine_barrier()

    TileContext._drain_and_barrier = _patched


_patch_tile_drain()


# revision 18
# speedup vs baseline: 6.0533x; 1.6800x over previous
"""DCNv2 (spatially-constant offsets) Trainium2 Bass kernel, 8-core SPMD.

Math: out[B,g*16+o,i,j] = sum_{ky,kx,c} w[g,o,c,ky,kx] * smp
     smp = bilinear sample of x[B//2, g*3+c] at (i + dy(ky), j + dx(kx)),
     dy = p[ky]*(1+3/off_y), dx = p[kx]*(1+3/off_x), p = [-1,0,1],
     zero padding outside the image.

Separable structure: integer y-shift + y-fraction folded into a per-partition
row GATHER (indirect DMA) over a zero-padded input; integer x-shift +
x-fraction folded into a small set of PSUM-accumulating matmuls whose rhs
reads the gathered rows at a per-pass column offset delta, with host-folded
block-diagonal weights (data, so the single SPMD program works for all
cores' different offsets).

Sharding: off_b (16) split 2-per-core across 8 cores (core i handles
off_b {2i, 2i+1}, which both read input batch i).
"""

import os
import sys

sys.path.insert(0, "/opt/trn_rl_repo")

import ml_dtypes
import numpy as np

import concourse.bass as bass
import concourse.bacc as bacc
import concourse.mybir as mybir
from concourse.tile import TileContext
from concourse.bass_utils import run_bass_kernel_spmd

# ---- fixed problem geometry (hardcoded per task rules) ----
KS = 3
H = W = 160
PAD = 5
HP = WP = H + 2 * PAD  # 170
CH = 9                  # channels per input batch (num_sq*ct)
G = 3                   # groups
CG = 3                  # channels per group
COUT = 48
OG = COUT // G          # 16 outputs per group
NCORES = 8
NPAIR = 6               # (2 off_b) x (3 groups) per core
# y-tap slots (ky, a): ky=1 is the center (dy=0 -> single tap)
YT = [(0, 0), (0, 1), (1, 0), (2, 0), (2, 1)]
NT = len(YT)            # 5
KPART = NPAIR * NT * CG  # 90 live contraction partitions
KP = 128                 # padded to 128 so bf16 fast-weight-load engages
M = NPAIR * OG           # 96 output partitions
PR = np.array([-1.0, 0.0, 1.0], dtype=np.float64)

# fat-row gather table: channel rows padded to HPAD, strip height SH rows per
# gathered unit, one alignment copy per possible y-start s in [1, 9]
SH = 32
HPAD = 192               # 6*SH, multiple of SH covering HP=170
NS = 9                   # alignment copies (start = i0 + 5 + yshift, yshift in [-4,4])
FATROWS = (CH * HPAD - SH) // SH  # fat rows per alignment copy = 53
FATW = SH * WP           # 5440 elements per fat row

_prog_cache = {}


# ---------------------------------------------------------------- device code
def _build_program(ndelta, deltas):
    """One SPMD program; per-core variation only through tensor data."""
    nc = bacc.Bacc("TRN2", target_bir_lowering=False, debug=False,
                   num_devices=NCORES)
    xtab = nc.declare_dram_parameter("xtab", [NS * FATROWS + 1, FATW],
                                     mybir.dt.bfloat16, isOutput=False)
    gidx = nc.declare_dram_parameter("gidx", [KP, H // SH], mybir.dt.int32,
                                     isOutput=False)
    wfold = nc.declare_dram_parameter("wfold", [KP, ndelta * M],
                                      mybir.dt.bfloat16, isOutput=False)
    y = nc.declare_dram_parameter("y", [M, H, W], mybir.dt.float32,
                                  isOutput=True)

    nstrips = H // SH

    with TileContext(nc) as tc:
        with (
            tc.tile_pool(name="const", bufs=1) as cpool,
            tc.tile_pool(name="gat", bufs=3) as gpool,
            tc.tile_pool(name="ps", bufs=1, space="PSUM") as ppool,
            tc.tile_pool(name="ost", bufs=4) as opool,
        ):
            idx_sb = cpool.tile([KP, nstrips], mybir.dt.int32, tag="idx")
            nc.sync.dma_start(idx_sb[:], gidx[:])
            w_sb = cpool.tile([KP, ndelta * M], mybir.dt.bfloat16, tag="w")
            nc.sync.dma_start(w_sb[:], wfold[:])

            # i-triples per strip, grouped in chunks of <=4 so LDWEIGHTS is
            # amortized over the chunk (delta is the outer loop per chunk)
            triples = [(il, min(3, SH - il)) for il in range(0, SH, 3)]
            chunks = [triples[i:i + 4] for i in range(0, len(triples), 4)]

            for s in range(nstrips):
                i0 = s * SH
                # one fat-row gather per strip: partition p receives SH
                # consecutive padded rows starting at its (y-shifted) base;
                # pad partitions (>=KPART) point at the all-zero fat row
                gt = gpool.tile([KP, SH, WP], mybir.dt.bfloat16, tag="g")
                nc.gpsimd.indirect_dma_start(
                    out=gt.rearrange("p a b -> p (a b)"),
                    out_offset=None,
                    in_=xtab[:],
                    in_offset=bass.IndirectOffsetOnAxis(
                        ap=idx_sb[:, s:s + 1], axis=0),
                )
                for chunk in chunks:
                    pts = {}
                    for il, rows in chunk:
                        pts[il] = ppool.tile([M, 512], mybir.dt.float32,
                                             name=f"pt_{s}_{il}",
                                             tag=f"p{il // 3 % 8}")
                    for d in range(ndelta):
                        off = PAD + deltas[d]
                        for il, rows in chunk:
                            nc.tensor.matmul(
                                pts[il][:, :rows * W],
                                w_sb[:, d * M:(d + 1) * M],
                                gt[:, il:il + rows, off:off + W],
                                start=(d == 0),
                                stop=(d == ndelta - 1),
                            )
                    for il, rows in chunk:
                        n = rows * W
                        ot = opool.tile([M, 512], mybir.dt.float32, tag="o")
                        nc.scalar.copy(ot[:, :n], pts[il][:, :n])
                        nc.sync.dma_start(
                            y[:, i0 + il:i0 + il + rows, :], ot[:, :n])
    nc.finalize()
    return nc


# ------------------------------------------------------------------ host prep
def _fold(inputs):
    """Per-core in_maps + the program-wide delta list."""
    x = np.asarray(inputs["input"], dtype=np.float32)    # (8,1,9,160,160)
    wt = np.asarray(inputs["weight"], dtype=np.float32)  # (3,3,48,3)
    off = np.asarray(inputs["offset"], dtype=np.float64)  # (16,3,2)

    # wmat[g, o, c, k]  (k = ky*3+kx)
    wmat = wt.transpose(2, 3, 0, 1).reshape(G, OG, CG, KS * KS)

    d_y = 1.0 + KS / off[:, :, 0]   # (16,3)
    d_x = 1.0 + KS / off[:, :, 1]
    dy = PR[None, None, :] * d_y[:, :, None]   # (16,3,ky)
    dx = PR[None, None, :] * d_x[:, :, None]
    oy = np.floor(dy).astype(np.int64)
    ox = np.floor(dx).astype(np.int64)
    wy = dy - oy
    wx = dx - ox

    # program-wide union of x-column offsets (delta = ox + b)
    dset = set()
    for B in range(off.shape[0]):
        for g in range(G):
            for kx in range(KS):
                dset.add(int(ox[B, g, kx]))
                if wx[B, g, kx] != 0.0:
                    dset.add(int(ox[B, g, kx]) + 1)
    deltas = sorted(dset)
    nd = len(deltas)
    dpos = {d: i for i, d in enumerate(deltas)}

    nstrips = H // SH
    in_maps = []
    for core in range(NCORES):
        xc = x[core, 0]  # (9,160,160)
        xp = np.zeros((CH, HPAD, WP), dtype=ml_dtypes.bfloat16)
        xp[:, PAD:PAD + H, PAD:PAD + W] = xc.astype(ml_dtypes.bfloat16)
        xflat = xp.reshape(CH * HPAD, WP)
        xtab = np.concatenate([
            np.stack([
                xflat[s:s + FATROWS * SH].reshape(FATROWS, FATW)
                for s in range(1, NS + 1)
            ]).reshape(NS * FATROWS, FATW),
            np.zeros((1, FATW), dtype=ml_dtypes.bfloat16),
        ])

        # pad partitions [KPART, KP) gather the all-zero fat row and carry
        # zero weights
        gi = np.full((KP, nstrips), NS * FATROWS, dtype=np.int32)
        wf = np.zeros((KP, nd, M), dtype=np.float64)
        for p in range(2):
            B = 2 * core + p
            for g in range(G):
                q = p * G + g
                for t, (ky, a) in enumerate(YT):
                    cy = (1.0 - wy[B, g, ky]) if a == 0 else wy[B, g, ky]
                    s_al = PAD + int(oy[B, g, ky]) + a  # in [1, 9]
                    for c in range(CG):
                        P = (q * NT + t) * CG + c
                        ch = g * CG + c
                        for st in range(nstrips):
                            gi[P, st] = ((s_al - 1) * FATROWS
                                         + (ch * HPAD + st * SH) // SH)
                        for kx in range(KS):
                            for b in range(2):
                                cx = (1.0 - wx[B, g, kx]) if b == 0 \
                                    else wx[B, g, kx]
                                if cx == 0.0:
                                    continue
                                d = dpos[int(ox[B, g, kx]) + b]
                                k = ky * KS + kx
                                wf[P, d, q * OG:(q + 1) * OG] += (
                                    cy * cx * wmat[g, :, c, k])
        in_maps.append({
            "xtab": xtab,
            "gidx": gi,
            "wfold": wf.reshape(KP, nd * M).astype(ml_dtypes.bfloat16),
        })
    return in_maps, deltas


def kernel(**inputs):
    in_maps, deltas = _fold(inputs)
    nd = len(deltas)
    key = (nd, tuple(deltas))
    if key not in _prog_cache:
        _prog_cache[key] = _build_program(nd, deltas)
    nc = _prog_cache[key]

    trace = bool(int(os.environ.get("BASSDCN_TRACE", "0")))
    if trace:
        _install_ntff_hook()
    res = run_bass_kernel_spmd(nc, in_maps, list(range(NCORES)), trace=trace)
    if trace:
        kernel.last_exec_time_ns = res.exec_time_ns
        kernel.last_results = res

    out = np.empty((16, COUT, H, W), dtype=np.float32)
    for core in range(NCORES):
        yc = res.results[core]["y"].reshape(2, G, OG, H, W)
        out[2 * core] = yc[0].reshape(COUT, H, W)
        out[2 * core + 1] = yc[1].reshape(COUT, H, W)
    return out


def _install_ntff_hook():
    """The agent image's antenv lacks axon_hooks; synthesize it so
    run_bass_kernel_spmd(trace=True) can NTFF-profile via libaxon_pjrt."""
    import types
    try:
        import antenv.axon_hooks  # noqa: F401
        return
    except ImportError:
        pass
    try:
        sys.path.insert(0, "/root/.axon_site")
        from trn_agent_boot.trn_boot import _ntff_profile_via_ctypes
        hook = _ntff_profile_via_ctypes("/opt/axon/libaxon_pjrt.so")
    except Exception:
        hook = None
    m = types.ModuleType("antenv.axon_hooks")
    m.get_axon_ntff_profile_hook = lambda: hook
    m.set_axon_ntff_profile_hook = lambda h: None
    sys.modules["antenv.axon_hooks"] = m


# ------------------------------------------------- tile drain walrus workaround
def _patch_tile_drain():
    from bass_rust import ScopedClock

    def _patched(self, tick_clock, wait_clock):
        nc = self.nc
        drain_inst = nc.sync.drain()
        wait_clock.add_sem_waits(
            drain_inst.ins, ScopedClock({None: tick_clock.global_clock}))
        si = drain_inst.ins.sync_info
        waits = list(si.on_wait or [])
        if len(waits) > 1:
            si.on_wait = waits[:1]
            drain_inst.ins.sync_info = si
            for w in waits[1:]:
                nop = nc.sync.nop(nofuse=True, hint="drain_wait_split")
                nsi = nop.ins.sync_info
                if nsi is None:
                    nsi = mybir.SyncInfo(on_wait=[w], on_update=[])
                else:
                    nsi.on_wait = [w]
                nop.ins.sync_info = nsi
        nc.all_engine_barrier()
        assert self.sems is not None
        popped = nc._tile_sem_poison_stack.pop()
        assert popped is self._sem_poison
        nc.clear_and_free_semaphores(list(self.sems.allocated().values()))
        nc.all_engine_barrier()

    TileContext._drain_and_barrier = _patched


_patch_tile_drain()
